# revision 3
# baseline (speedup 1.0000x reference)
"""MoE expert FFN (CachedKimiExperts) on 8 Trainium2 NeuronCores.

Expert-parallel: core c owns experts [2c, 2c+1].  Routing and token
gather/scatter run on the host.  Both w1 and w2 are stored in HBM as
int8 with per-input-feature symmetric scales (HBM stream 12.6MB/core
+ ~2MB IO).

Scale folding (all free):
  - w1's scale s1[e,h] multiplies the gathered tokens ON THE HOST.
  - w2's scale s2[e,i] rides the actT PSUM->SBUF copy as a
    per-partition tensor_scalar_mul (partitions = i).

DMA ring assignment (each HWDGE ring is a FIFO; mixing streams
starves the critical one):
  - SP ring   (nc.sync):   the w1 int8 stream, nothing else.
  - ACT ring  (nc.scalar): xg, s2, w2q prefetches.
  - SWDGE     (nc.gpsimd): y output writes only.

int8->fp16 casts run on vector+scalar with a projected-finish-time
balancer (HW rates: v 0.57ns/elem-pp, a 0.93ns/elem-pp); the next
block's first four w1 tiles are cast during this block's down phase
(issued right after the act section), because cast supply
(~1.4 slices/us) cannot keep up with gate/up demand (~2.3/us) in
real time.  One w2 slice per expert goes to the otherwise-idle
gpsimd engine.

PSUM layout (exactly 8 banks):
  gate/warm [P,1024]f32  2 banks (tag gate; HAM-warmup matmuls share
            this tag so they are forced BEFORE the first real matmul)
  up/yA/yB  [P,1024]f32  4 banks (tag upy, bufs=2 ring)
  tp        [P,128]f16   2 banks (bufs=2 ping-pong)
"""

import os
import sys

import numpy as np

for _p in ("/opt/trn_rl_repo", "/root/.axon_site/_ro/trn_rl_repo"):
    if os.path.isdir(_p) and _p not in sys.path:
        sys.path.append(_p)

import concourse.bass as bass  # noqa: F401  (bass must import before tile)
import concourse.mybir as mybir
import concourse.tile as tile
from concourse import bacc, bass_utils
from concourse.masks import make_identity

N_CORES = 8
E = 16
E_LOC = E // N_CORES  # experts per core
H = 2048  # hidden dim
I = 1024  # expert intermediate dim
I2 = 2 * I  # fused gate+up width
P = 128  # partitions
FD = 512  # matmul moving free dim (one fp32 PSUM bank)
KC1 = H // P  # k-chunks for the gate/up matmul (contract over H)
KC2 = I // P  # k-chunks for the down matmul (contract over I)
G1 = KC1 // 2  # w1 tile groups (2 k-chunks per tile)

F16 = mybir.dt.float16
F32 = mybir.dt.float32
I8 = mybir.dt.int8

WARMUP = 40

TRACE = False
TRACE_CORES = None
LAST_RESULTS = None

_programs = {}


class _CastBalancer:
    """Projected-finish-time engine picker for int8->fp16 cast ops.

    Rates in ns/elem-per-partition measured from HW traces.  Non-cast
    work (silu, act muls, actT scale-copies, y copies, dma issues) is
    reported via other() as it is issued."""

    def __init__(self, nc):
        self.nc = nc
        self.load = {"v": 0.0, "a": 2600.0}  # scalar preloads act tables
        self.rate = {"v": 0.59, "a": 0.80}
        self.fix = {"v": 80.0, "a": 360.0}

    def other(self, eng, ns):
        self.load[eng] += ns

    def cast(self, out_ap, in_ap, elems, eng=None):
        if eng is None:
            eng = min(
                self.load,
                key=lambda k: self.load[k] + elems * self.rate[k] + self.fix[k],
            )
        self.load[eng] += elems * self.rate[eng] + self.fix[eng]
        if eng == "v":
            self.nc.vector.tensor_copy(out_ap, in_ap)
        else:
            self.nc.scalar.copy(out_ap, in_ap)


def _build_program(C):
    """Bass/Tile program for one core: E_LOC experts x (C tokens each)."""
    CB = C // P  # token blocks per expert
    n_blocks = E_LOC * CB

    nc = bacc.Bacc(
        "TRN2", target_bir_lowering=False, debug=False, num_devices=N_CORES
    )
    # w1q[e, g, p, j, :] = int8 quant of w1[2c+e].T[(2g+j)*128 + p, :]
    w1q = nc.dram_tensor("w1q", [E_LOC, G1, P, 2, I2], I8, kind="ExternalInput")
    # w2q[e, p, ic, :] = int8 quant of w2[2c+e].T[ic*128 + p, :]
    w2q = nc.dram_tensor("w2q", [E_LOC, P, KC2, H], I8, kind="ExternalInput")
    # s2[e, p, ic] = dequant scale of w2 input-feature ic*128 + p
    s2d = nc.dram_tensor("s2", [E_LOC, P, KC2], F32, kind="ExternalInput")
    # xg[e, p, kc, c] = (x * s1[e]).T[kc*128 + p, tok_c(e)]  (gathered, padded)
    xg = nc.dram_tensor("xg", [E_LOC, P, KC1, C], F16, kind="ExternalInput")
    y = nc.dram_tensor("y", [E_LOC, C, H], F16, kind="ExternalOutput")

    blocks = [(e, cb) for e in range(E_LOC) for cb in range(CB)]

    with tile.TileContext(nc) as tc:
        with (
            tc.tile_pool(name="w1qpool", bufs=10) as w1qpool,
            tc.tile_pool(name="w1fpool", bufs=6) as w1fpool,
            tc.tile_pool(name="w2qpool", bufs=1) as w2qpool,
            tc.tile_pool(name="w2fpool", bufs=1) as w2fpool,
            tc.tile_pool(name="s2pool", bufs=min(n_blocks, 2)) as s2pool,
            tc.tile_pool(name="xp", bufs=2) as xp,
            tc.tile_pool(name="actp", bufs=2) as actp,
            tc.tile_pool(name="yp", bufs=4) as yp,
            tc.tile_pool(name="constp", bufs=1) as constp,
            tc.tile_pool(name="ps", bufs=1, space="PSUM") as ps,
        ):
            bal = _CastBalancer(nc)
            ident = constp.tile([P, P], F16, name="ident")
            make_identity(nc, ident)

            # HAM warmup on the "gate" PSUM tag: real gate matmuls then
            # WAR-depend on these, pinning them FIRST in the PE stream.
            warm_ps = ps.tile([P, P], F32, tag="gate", bufs=1, name="warm_ps")
            for _ in range(WARMUP):
                nc.tensor.matmul(warm_ps, ident, ident, start=True, stop=True)

            def load_w1_tile(e, g, split_first=False, eng=None):
                """DMA an int8 w1 tile (SP ring) + engine-cast to fp16."""
                wq = w1qpool.tile([P, 2, I2], I8, tag="wq", name="wq")
                if split_first:
                    nc.sync.dma_start(wq[:, 0, :], w1q[e, g, :, 0, :])
                    nc.sync.dma_start(wq[:, 1, :], w1q[e, g, :, 1, :])
                else:
                    nc.sync.dma_start(wq, w1q[e, g])
                wf = w1fpool.tile([P, 2, I2], F16, tag="wf", name="wf")
                if split_first:
                    hn = I2 // 2
                    for j in range(2):
                        bal.cast(wf[:, j, :hn], wq[:, j, :hn], hn, eng)
                        bal.cast(wf[:, j, hn:], wq[:, j, hn:], hn, eng)
                else:
                    bal.cast(wf[:, 0, :], wq[:, 0, :], I2, eng)
                    bal.cast(wf[:, 1, :], wq[:, 1, :], I2, eng)
                return wf

            xg_ts = {}
            s2sb = {}
            w2qt_sb = {}
            w2sb = {}

            def prefetch_block(b):
                """xg/s2 for block b via the ACT ring (scalar HWDGE)."""
                if b >= n_blocks:
                    return
                e, cb = blocks[b]
                xg_t = xp.tile([P, KC1, P], F16, tag="xg", name="xg_t")
                cs = slice(cb * P, (cb + 1) * P)
                hk = KC1 // 2
                nc.scalar.dma_start(xg_t[:, :hk, :], xg[e, :, :hk, cs])
                nc.scalar.dma_start(xg_t[:, hk:, :], xg[e, :, hk:, cs])
                bal.other("a", 1200)
                xg_ts[b] = xg_t
                if cb == 0:
                    s2t = s2pool.tile([P, KC2], F32, tag="s2", name="s2t")
                    nc.scalar.dma_start(s2t, s2d[e])
                    bal.other("a", 600)
                    s2sb[e] = s2t

            def fetch_w2q(e):
                """w2 int8 prefetch via the ACT ring."""
                if e in w2qt_sb:
                    return
                w2qt = w2qpool.tile([P, KC2, H], I8, tag="w2q", name="w2qt")
                hc = KC2 // 2
                nc.scalar.dma_start(w2qt[:, :hc, :], w2q[e, :, :hc, :])
                nc.scalar.dma_start(w2qt[:, hc:, :], w2q[e, :, hc:, :])
                bal.other("a", 1200)
                w2qt_sb[e] = w2qt

            def cast_w2_ic(e, ic):
                """Engine-cast one i-chunk of w2[e]; the last chunk goes
                to the otherwise-idle gpsimd engine."""
                if e not in w2sb:
                    w2sb[e] = w2fpool.tile(
                        [P, KC2, H], F16, tag="w2f", name="w2f"
                    )
                bal.cast(w2sb[e][:, ic, :], w2qt_sb[e][:, ic, :], H)

            wf_pre = {}
            for b, (e, cb) in enumerate(blocks):
                if b == 0:
                    wf_pre[(0, 0)] = load_w1_tile(e, 0, split_first=True)
                    wf_pre[(0, 1)] = load_w1_tile(e, 1)
                    prefetch_block(0)
                    fetch_w2q(e)
                xg_t = xg_ts[b]

                # ---- gate/up projection: h[c, i2] = x @ w1[e].T ----
                gate_ps = ps.tile([P, I], F32, tag="gate", bufs=1,
                                  name="gate_ps")
                up_ps = ps.tile([P, I], F32, tag="upy", bufs=2, name="up_ps")
                for g in range(G1):
                    wf = wf_pre.pop((b, g), None)
                    if wf is None:
                        wf = load_w1_tile(e, g)
                    if b == 0 and 2 <= g <= 4:
                        # block0's gate/up is cast-supply paced; fillers
                        # keep the HAM activity window alive through the
                        # stalls so real matmuls run at 2.4GHz
                        for _ in range(2):
                            fill_g = ps.tile([P, P], F32, tag="tp", bufs=2,
                                             name="fill_g")
                            nc.tensor.matmul(fill_g, ident, ident,
                                             start=True, stop=True)
                    for j in range(2):
                        kc = 2 * g + j
                        lhsT = xg_t[:, kc, :]
                        st = kc == 0
                        sp = kc == KC1 - 1
                        for out_ap, rhs in (
                            (gate_ps[:, :FD], wf[:, j, 0:FD]),
                            (gate_ps[:, FD:], wf[:, j, FD:I]),
                            (up_ps[:, :FD], wf[:, j, I:I + FD]),
                            (up_ps[:, FD:], wf[:, j, I + FD:]),
                        ):
                            nc.tensor.matmul(out_ap, lhsT, rhs,
                                             start=st, stop=sp)

                if b + 1 < n_blocks:
                    prefetch_block(b + 1)
                    fetch_w2q(blocks[b + 1][0])

                # PE fillers: the silu/transpose handoff leaves a ~2-3us
                # PE hole; a few ident-matmuls (no readers) keep the HAM
                # activity window alive so the down matmuls run at 2.4GHz
                for _ in range(4):
                    fill_t = ps.tile([P, P], F32, tag="tp", bufs=2,
                                     name="fill_t")
                    nc.tensor.matmul(fill_t, ident, ident,
                                     start=True, stop=True)

                # ---- act = silu(gate) * up, cast to fp16 ----
                QW = I // 4
                sg = actp.tile([P, I], F32, tag="sg", name="sg")
                act = actp.tile([P, I], F16, tag="act", name="act")
                for q in range(4):
                    qs = slice(q * QW, (q + 1) * QW)
                    nc.scalar.activation(
                        sg[:, qs],
                        gate_ps[:, qs],
                        mybir.ActivationFunctionType.Silu,
                    )
                    bal.other("a", 475)
                    nc.vector.tensor_mul(act[:, qs], sg[:, qs], up_ps[:, qs])
                    bal.other("v", 420)

                # ---- transpose act, fold in w2's dequant scale ----
                actT = actp.tile([P, KC2, P], F16, tag="actT", name="actT")
                for ic in range(KC2):
                    tp_t = ps.tile([P, P], F16, tag="tp", bufs=2, name="tp_t")
                    nc.tensor.transpose(
                        tp_t, act[:, ic * P:(ic + 1) * P], ident
                    )
                    nc.vector.tensor_scalar_mul(
                        actT[:, ic, :], tp_t, s2sb[e][:, ic:ic + 1]
                    )
                    bal.other("v", 280)

                # this expert's w2 casts + the next block's first four
                # w1 tiles run during the down phase, when gate/up cast
                # demand is zero (the down matmuls only need actT+w2f)
                if cb == 0:
                    for ic in range(KC2):
                        cast_w2_ic(e, ic)
                for g in range(4):
                    if b + 1 < n_blocks:
                        wf_pre[(b + 1, g)] = load_w1_tile(blocks[b + 1][0], g)

                # ---- down projection in two half-H passes ----
                w2f = w2sb[e]
                for half in range(2):
                    y_ps = ps.tile([P, I], F32, tag="upy", bufs=2,
                                   name="y_ps")
                    hb = half * I
                    for ic in range(KC2):
                        st = ic == 0
                        sp = ic == KC2 - 1
                        nc.tensor.matmul(y_ps[:, :FD], actT[:, ic, :],
                                         w2f[:, ic, hb:hb + FD],
                                         start=st, stop=sp)
                        nc.tensor.matmul(y_ps[:, FD:], actT[:, ic, :],
                                         w2f[:, ic, hb + FD:hb + I],
                                         start=st, stop=sp)
                    # y copies: vector takes psum bank 0 (chunks 0,1),
                    # scalar bank 1 (chunks 2,3); DMA via SWDGE (gpsimd,
                    # otherwise idle -- keeps both HWDGE rings clean)
                    last = b == n_blocks - 1 and half == 1
                    if last:
                        # tail: 2 fat chunks, copies on v/a in parallel,
                        # DMAs on the empty SP ring
                        y_sb0 = yp.tile([P, FD], F16, tag="ysbw",
                                        name="y_sb0", bufs=2)
                        nc.scalar.copy(y_sb0, y_ps[:, FD:])
                        y_sb1 = yp.tile([P, FD], F16, tag="ysbw",
                                        name="y_sb1", bufs=2)
                        nc.vector.tensor_copy(y_sb1, y_ps[:, :FD])
                        nc.sync.dma_start(
                            y[e, cb * P:(cb + 1) * P, hb + FD:hb + I], y_sb0
                        )
                        nc.sync.dma_start(
                            y[e, cb * P:(cb + 1) * P, hb:hb + FD], y_sb1
                        )
                        continue
                    YQ = I // 4
                    y_sbs = []
                    for hh in (2, 3):
                        y_sb = yp.tile([P, YQ], F16, tag="ysb", name="y_sb")
                        nc.scalar.copy(y_sb, y_ps[:, hh * YQ:(hh + 1) * YQ])
                        bal.other("a", 480)
                        y_sbs.append((hh, y_sb))
                    for hh in (0, 1):
                        y_sb = yp.tile([P, YQ], F16, tag="ysb", name="y_sb")
                        nc.vector.tensor_copy(
                            y_sb, y_ps[:, hh * YQ:(hh + 1) * YQ]
                        )
                        bal.other("v", 240)
                        y_sbs.append((hh, y_sb))
                    y_eng = nc.sync if b == n_blocks - 1 else nc.gpsimd
                    for hh, y_sb in y_sbs:
                        y_eng.dma_start(
                            y[
                                e,
                                cb * P:(cb + 1) * P,
                                hb + hh * YQ:hb + (hh + 1) * YQ,
                            ],
                            y_sb,
                        )
    nc.finalize()
    return nc


def _route(router_logits, top_k):
    """softmax -> top-k -> renormalize; per-expert token lists + weights."""
    lg = np.asarray(router_logits, dtype=np.float64)
    T, num_e = lg.shape
    k = int(np.asarray(top_k))
    p = np.exp(lg - lg.max(axis=-1, keepdims=True))
    p /= p.sum(axis=-1, keepdims=True)
    idx = np.argpartition(-p, k - 1, axis=1)[:, :k]  # [T, k] top-k set
    vals = np.take_along_axis(p, idx, axis=1)
    wts = vals / vals.sum(axis=-1, keepdims=True)
    tok_idx = [[] for _ in range(num_e)]
    tok_w = [[] for _ in range(num_e)]
    for t in range(T):
        for j in range(k):
            tok_idx[idx[t, j]].append(t)
            tok_w[idx[t, j]].append(wts[t, j])
    return tok_idx, tok_w


def _quant_int8(wt):
    """Symmetric per-input-feature int8 quant of [E_LOC, K, N] (axis=2).

    Returns (q int8 [E_LOC, K, N], scale fp64 [E_LOC, K])."""
    s = np.max(np.abs(wt), axis=2) / 127.0
    s = np.maximum(s, 1e-30)
    q = np.clip(np.round(wt / s[:, :, None]), -127, 127).astype(np.int8)
    return q, s


def kernel(x, router_logits, w1, w2, top_k):
    global LAST_RESULTS
    x = np.asarray(x)
    w1 = np.asarray(w1)
    w2 = np.asarray(w2)
    T = x.shape[0]

    tok_idx, tok_w = _route(router_logits, top_k)
    max_count = max(max(len(ti) for ti in tok_idx), 1)
    C = ((max_count + P - 1) // P) * P

    prog = _programs.get(C)
    if prog is None:
        prog = _programs[C] = _build_program(C)

    xT = np.asarray(x, dtype=np.float64).T  # [H, T]
    in_maps = []
    for c in range(N_CORES):
        sl = slice(c * E_LOC, (c + 1) * E_LOC)
        # [E_LOC, H, 2I] int8-quantized, per-input-feature h scales
        w1tc = w1[sl].transpose(0, 2, 1).astype(np.float64)
        w1qv, s1 = _quant_int8(w1tc)  # q:[E_LOC, H, 2I], s1:[E_LOC, H]
        w1qc = np.ascontiguousarray(
            w1qv.reshape(E_LOC, G1, 2, P, I2).transpose(0, 1, 3, 2, 4)
        )
        # [E_LOC, I, H] int8-quantized, per-input-feature i scales
        w2tc = w2[sl].transpose(0, 2, 1).astype(np.float64)
        w2qv, s2v = _quant_int8(w2tc)  # q:[E_LOC, I, H], s2:[E_LOC, I]
        w2qc = np.ascontiguousarray(
            w2qv.reshape(E_LOC, KC2, P, H).transpose(0, 2, 1, 3)
        )
        s2c = np.ascontiguousarray(
            s2v.reshape(E_LOC, KC2, P).transpose(0, 2, 1).astype(np.float32)
        )
        # gathered tokens, pre-scaled by w1's dequant scale s1[e, h]
        xgc = np.zeros((E_LOC, P, KC1, C), np.float16)
        for el in range(E_LOC):
            ti = tok_idx[c * E_LOC + el]
            if ti:
                xs = (xT[:, ti] * s1[el][:, None]).astype(np.float16)
                xgc[el, :, :, : len(ti)] = (
                    xs.reshape(KC1, P, len(ti)).transpose(1, 0, 2)
                )
        in_maps.append({"w1q": w1qc, "w2q": w2qc, "s2": s2c, "xg": xgc})

    LAST_RESULTS = bass_utils.run_bass_kernel_spmd(
        prog,
        in_maps,
        core_ids=list(range(N_CORES)),
        trace=TRACE,
        trace_cores=TRACE_CORES,
    )

    out = np.zeros((T, H), dtype=np.float64)
    for c in range(N_CORES):
        yv = LAST_RESULTS.results[c]["y"]  # [E_LOC, C, H] fp16
        for el in range(E_LOC):
            ge = c * E_LOC + el
            ti = tok_idx[ge]
            if ti:
                wv = np.asarray(tok_w[ge], dtype=np.float64)[:, None]
                out[ti] += wv * yv[el][: len(ti)].astype(np.float64)
    return out.astype(x.dtype)
